# revision 19
# baseline (speedup 1.0000x reference)
"""Trainium2 Bass kernel for nn_BinClassDecoder (Bahdanau additive-attention
binary classifier decoder).

Contract: kernel(**inputs) takes the FULL unsharded inputs (numpy arrays, keys
as in reference.setup_inputs()) and returns the FULL [B, T, 1] float32 output.

Sharding: 8 NeuronCores; core c computes t-positions [8c, 8c+8) for ALL
batches (perfectly balanced in the dominant [B,t,s,d] tanh work even though
memory_lengths vary per batch).  The s-dimension is truncated per batch to
Lb = ceil(len_b/128)*128 (everything past len_b is softmax-masked to 0, so
truncation is exact).
"""

import math
import os

import numpy as np

B, S, T = 8, 512, 64
ENC, WORD = 512, 512
NCORES = 8
TL = T // NCORES  # t-positions per core = 8
NEG = -1.0e30

BF16 = None  # filled lazily (ml_dtypes)


def _ceil128(x):
    return int(min(max(int(math.ceil(x / 128.0)) * 128, 128), 512))


# ---------------------------------------------------------------------------
# device kernel builder
# ---------------------------------------------------------------------------

def _build_nc(Lb, debug=False):
    """Build + bacc-compile the Bass program for per-batch truncated lengths
    Lb (tuple of 8 ints, each a multiple of 128 in [128, 512])."""
    import concourse.bass as bass
    import concourse.tile as tile
    from concourse import bacc, mybir

    f32 = mybir.dt.float32
    bf16 = mybir.dt.bfloat16

    Lb = list(Lb)
    cum = [0]
    for b in range(B):
        cum.append(cum[-1] + Lb[b])
    SL = cum[-1]                      # total truncated s-cols across batches
    Sb = [l // 128 for l in Lb]       # 128-chunks per batch
    cumS = [0]
    for b in range(B):
        cumS.append(cumS[-1] + Sb[b])
    NS = cumS[-1]

    nc = bacc.Bacc()

    # ---------------- DRAM inputs -----------------
    d_mbT = nc.dram_tensor("mbT", [4, 128, SL], bf16, kind="ExternalInput")
    d_mbN = nc.dram_tensor("mbN", [NS, 128, ENC], f32, kind="ExternalInput")
    d_wcT = nc.dram_tensor("wcT", [4, 128, ENC], bf16, kind="ExternalInput")
    d_wqT = nc.dram_tensor("wqT", [4, 128, ENC], f32, kind="ExternalInput")
    d_wcwT = nc.dram_tensor("wcwT", [4, 128, WORD], f32, kind="ExternalInput")
    d_wecT = nc.dram_tensor("wecT", [4, 128, WORD], f32, kind="ExternalInput")
    d_weoT = nc.dram_tensor("weoT", [8, 128, WORD], f32, kind="ExternalInput")
    d_tg = nc.dram_tensor("tg", [4, 128, 64], f32, kind="ExternalInput")
    d_eh = nc.dram_tensor("eh", [8, 128, 64], f32, kind="ExternalInput")
    d_vsh = nc.dram_tensor("vsh", [4, 128, 63], bf16, kind="ExternalInput")
    d_vr = nc.dram_tensor("vr", [128, 4], f32, kind="ExternalInput")
    d_bq = nc.dram_tensor("bq", [128, 4], f32, kind="ExternalInput")
    d_bw = nc.dram_tensor("bw", [128, 4], f32, kind="ExternalInput")
    d_nbv = nc.dram_tensor("nbv", [1, 1], f32, kind="ExternalInput")
    d_msk = nc.dram_tensor("msk", [64, 512], f32, kind="ExternalInput")
    d_id = nc.dram_tensor("id64", [64, 64], f32, kind="ExternalInput")
    d_out = nc.dram_tensor("scores", [1, 64], f32, kind="ExternalOutput")
    if debug:
        d_dbg_wqb = nc.dram_tensor("dbg_wqb", [128, 4, 64], f32, kind="ExternalOutput")
        d_dbg_uh0 = nc.dram_tensor("dbg_uh0", [128, 4, Lb[0]], f32, kind="ExternalOutput")
        d_dbg_strip = nc.dram_tensor("dbg_strip", [128, 4, Lb[0]], f32, kind="ExternalOutput")
        d_dbg_align = nc.dram_tensor("dbg_align", [64, 512], f32, kind="ExternalOutput")
        d_dbg_A = nc.dram_tensor("dbg_A", [64, 512], f32, kind="ExternalOutput")
        d_dbg_AT = nc.dram_tensor("dbg_AT", [128, 4, 64], f32, kind="ExternalOutput")
        d_dbg_cT = nc.dram_tensor("dbg_cT", [128, 4, 64], f32, kind="ExternalOutput")
        d_dbg_ov = nc.dram_tensor("dbg_ov", [128, 4, 64], f32, kind="ExternalOutput")

    Tanh = mybir.ActivationFunctionType.Tanh
    Exp = mybir.ActivationFunctionType.Exp

    with tile.TileContext(nc) as tc:
        with (
            tc.tile_pool(name="consts", bufs=1) as consts,
            tc.tile_pool(name="work", bufs=1) as work,
            tc.tile_pool(name="strips", bufs=6) as strips,
            tc.tile_pool(name="ps_uh", bufs=2, space="PSUM") as ps_uh_pool,
            tc.tile_pool(name="ps_misc", bufs=1, space="PSUM") as ps_misc,
        ):
            # ---------- constant loads (rough order of first use) ----------
            sb_tg = consts.tile([128, 4, 64], f32)
            nc.sync.dma_start(out=sb_tg, in_=d_tg.rearrange("a p j -> p a j"))
            sb_wqT = consts.tile([128, 4, ENC], f32)
            nc.sync.dma_start(out=sb_wqT, in_=d_wqT.rearrange("a p d -> p a d"))
            sb_bq = consts.tile([128, 4], f32)
            nc.sync.dma_start(out=sb_bq, in_=d_bq[:, :])
            sb_mbT = consts.tile([128, 4, SL], bf16)
            nc.sync.dma_start(out=sb_mbT, in_=d_mbT.rearrange("a p s -> p a s"))
            sb_wcT = consts.tile([128, 4, ENC], bf16)
            nc.sync.dma_start(out=sb_wcT, in_=d_wcT.rearrange("a p d -> p a d"))
            sb_vsh = consts.tile([128, 4, 63], bf16)
            nc.sync.dma_start(out=sb_vsh, in_=d_vsh.rearrange("a p c -> p a c"))
            sb_msk = consts.tile([64, 512], f32)
            nc.sync.dma_start(out=sb_msk, in_=d_msk[:, :])
            sb_id = consts.tile([64, 64], f32)
            nc.sync.dma_start(out=sb_id, in_=d_id[:, :])
            sb_mbN = consts.tile([128, NS, ENC], f32)
            nc.sync.dma_start(out=sb_mbN, in_=d_mbN.rearrange("a p d -> p a d"))
            sb_wcwT = consts.tile([128, 4, WORD], f32)
            nc.sync.dma_start(out=sb_wcwT, in_=d_wcwT.rearrange("a p d -> p a d"))
            sb_wecT = consts.tile([128, 4, WORD], f32)
            nc.sync.dma_start(out=sb_wecT, in_=d_wecT.rearrange("a p d -> p a d"))
            sb_weoT = consts.tile([128, 8, WORD], f32)
            nc.sync.dma_start(out=sb_weoT, in_=d_weoT.rearrange("a p d -> p a d"))
            sb_eh = consts.tile([128, 8, 64], f32)
            nc.sync.dma_start(out=sb_eh, in_=d_eh.rearrange("a p j -> p a j"))
            sb_vr = consts.tile([128, 4], f32)
            nc.sync.dma_start(out=sb_vr, in_=d_vr[:, :])
            sb_bw = consts.tile([128, 4], f32)
            nc.sync.dma_start(out=sb_bw, in_=d_bw[:, :])
            sb_nbv = consts.tile([1, 1], f32)
            nc.sync.dma_start(out=sb_nbv, in_=d_nbv[:, :])

            sb_zero = consts.tile([1, 576], f32)
            nc.vector.memset(sb_zero, 0.0)

            # ---------- wq projection: wqb[d, j] = (Wq h_t[j] + bq) ----------
            ps_wq = ps_misc.tile([128, 4, 64], f32, tag="psA")
            for dc in range(4):
                for kc in range(4):
                    nc.tensor.matmul(
                        ps_wq[:, dc, :],
                        sb_wqT[:, kc, dc * 128:(dc + 1) * 128],
                        sb_tg[:, kc, :],
                        start=(kc == 0),
                        stop=(kc == 3),
                    )
            wqb = work.tile([128, 4, 64], f32)
            for dc in range(4):
                nc.vector.tensor_scalar_add(
                    out=wqb[:, dc, :], in0=ps_wq[:, dc, :],
                    scalar1=sb_bq[:, dc:dc + 1],
                )
            if debug:
                nc.sync.dma_start(out=d_dbg_wqb[:, :, :], in_=wqb[:, :, :])

            # ---------- psum tile for align scores, cleared by a matmul ----
            ps_al = ps_misc.tile([128, 512], f32, tag="ps_al")
            nc.tensor.matmul(
                ps_al[0:64, :],
                sb_zero[0:1, 0:64],
                sb_zero[0:1, 0:512],
                start=True, stop=False, skip_group_check=True,
            )

            # ---------- word_hid + enc_hid (sequential groups per wc) ------
            ps_wv = ps_misc.tile([128, 4, 64], f32, tag="psB")
            for wc in range(4):
                for kc in range(4):
                    nc.tensor.matmul(
                        ps_wv[:, wc, :],
                        sb_wcwT[:, kc, wc * 128:(wc + 1) * 128],
                        sb_tg[:, kc, :],
                        start=(kc == 0), stop=False,
                        skip_group_check=True,
                    )
                for kc in range(8):
                    nc.tensor.matmul(
                        ps_wv[:, wc, :],
                        sb_weoT[:, kc, wc * 128:(wc + 1) * 128],
                        sb_eh[:, kc, :],
                        start=False, stop=(kc == 7),
                        skip_group_check=True,
                    )
            wv = work.tile([128, 4, 64], f32)
            nc.vector.tensor_copy(out=wv[:, :, :], in_=ps_wv[:, :, :])

            # ---------- per-batch: uh projection, then strips over t -------
            uh_tiles = []
            for b in range(B):
                L = Lb[b]
                uh_b = work.tile([128, 4, L], bf16, tag=f"uh{b}")
                uh_tiles.append(uh_b)
                for dc in range(4):
                    ps = ps_uh_pool.tile([128, 512], f32, tag="ps_uh")
                    for kc in range(4):
                        nc.tensor.matmul(
                            ps[:, 0:L],
                            sb_wcT[:, kc, dc * 128:(dc + 1) * 128],
                            sb_mbT[:, kc, cum[b]:cum[b] + L],
                            start=(kc == 0),
                            stop=(kc == 3),
                        )
                    nc.vector.tensor_copy(uh_b[:, dc, :], ps[:, 0:L])

                if debug and b == 0:
                    dbg_uh = work.tile([128, 4, L], f32, tag="dbg_uh")
                    nc.vector.tensor_copy(out=dbg_uh[:, :, :], in_=uh_b[:, :, :])
                    nc.sync.dma_start(out=d_dbg_uh0[:, :, :], in_=dbg_uh[:, :, :])

                for tl in range(TL):
                    j = b * TL + tl
                    strip = strips.tile([128, 4, Lb[b]], bf16, tag="strip")
                    for dc in range(4):
                        nc.vector.tensor_scalar_add(
                            out=strip[:, dc, :],
                            in0=uh_b[:, dc, :],
                            scalar1=wqb[:, dc, j:j + 1],
                        )
                    nc.scalar.activation(out=strip[:, :, :], in_=strip[:, :, :], func=Tanh)
                    if debug and b == 0 and tl == 0:
                        dbg_st = work.tile([128, 4, L], f32, tag="dbg_st")
                        nc.vector.tensor_copy(out=dbg_st[:, :, :], in_=strip[:, :, :])
                        nc.sync.dma_start(out=d_dbg_strip[:, :, :], in_=dbg_st[:, :, :])
                    pos = j % 32
                    blk = j // 32
                    last = (b == B - 1) and (tl == TL - 1)
                    for dc in range(4):
                        nc.tensor.matmul(
                            ps_al[32 * blk:32 * blk + 32, 0:L],
                            sb_vsh[:, dc, 31 - pos:63 - pos],
                            strip[:, dc, :],
                            start=False,
                            stop=(last and dc == 3),
                            skip_group_check=True,
                        )

            # ---------- masked softmax over s (no max-sub needed) ----------
            nc.vector.tensor_add(out=ps_al[0:64, :], in0=ps_al[0:64, :], in1=sb_msk)
            if debug:
                dbg_al = work.tile([64, 512], f32, tag="dbg_al")
                nc.vector.tensor_copy(out=dbg_al[:, :], in_=ps_al[0:64, :])
                nc.sync.dma_start(out=d_dbg_align[:, :], in_=dbg_al[:, :])
            A = work.tile([64, 512], f32)
            sums = work.tile([64, 1], f32)
            nc.scalar.activation(out=A, in_=ps_al[0:64, :], func=Exp, accum_out=sums)
            rec = work.tile([64, 1], f32)
            nc.vector.reciprocal(rec, sums)
            nc.vector.tensor_scalar_mul(out=A, in0=A, scalar1=rec)
            if debug:
                nc.sync.dma_start(out=d_dbg_A[:, :], in_=A[:, :])

            # ---------- A^T via PE transpose -------------------------------
            ps_at = ps_misc.tile([128, 4, 64], f32, tag="psA")
            for sc in range(4):
                nc.tensor.transpose(ps_at[:, sc, :], A[0:64, sc * 128:(sc + 1) * 128], sb_id)
            AT = work.tile([128, 4, 64], f32)
            nc.vector.tensor_copy(AT[:, :, :], ps_at[:, :, :])
            if debug:
                nc.sync.dma_start(out=d_dbg_AT[:, :, :], in_=AT[:, :, :])

            # ---------- context: cT[d, j] = sum_s h_s[s,d] A[j,s] ----------
            ps_ct = ps_misc.tile([128, 4, 64], f32, tag="psB")
            for b in range(B):
                for dc in range(4):
                    for sc in range(Sb[b]):
                        nc.tensor.matmul(
                            ps_ct[:, dc, b * 8:b * 8 + 8],
                            sb_mbN[:, cumS[b] + sc, dc * 128:(dc + 1) * 128],
                            AT[:, sc, b * 8:b * 8 + 8],
                            start=(sc == 0),
                            stop=(sc == Sb[b] - 1),
                            skip_group_check=True,
                        )
            cT = work.tile([128, 4, 64], f32)
            nc.vector.tensor_copy(cT[:, :, :], ps_ct[:, :, :])
            if debug:
                nc.sync.dma_start(out=d_dbg_cT[:, :, :], in_=cT[:, :, :])

            # ---------- cont_hid, add word+ench, tanh ----------------------
            ps_ov = ps_misc.tile([128, 4, 64], f32, tag="psC")
            ov = work.tile([128, 4, 64], f32)
            for wc in range(4):
                for kc in range(4):
                    nc.tensor.matmul(
                        ps_ov[:, wc, :],
                        sb_wecT[:, kc, wc * 128:(wc + 1) * 128],
                        cT[:, kc, :],
                        start=(kc == 0), stop=(kc == 3),
                        skip_group_check=True,
                    )
                nc.vector.tensor_add(
                    out=ps_ov[:, wc, :], in0=ps_ov[:, wc, :], in1=wv[:, wc, :],
                )
                nc.scalar.activation(
                    out=ov[:, wc, :], in_=ps_ov[:, wc, :], func=Tanh,
                    bias=sb_bw[:, wc:wc + 1],
                )

            # ---------- final score: sigmoid(ov . v_rank + b) --------------
            ps_sc = ps_misc.tile([128, 64], f32, tag="psC")
            for wc in range(4):
                nc.tensor.matmul(
                    ps_sc[0:1, :],
                    sb_vr[:, wc:wc + 1],
                    ov[:, wc, :],
                    start=(wc == 0),
                    stop=(wc == 3),
                )
            if debug:
                nc.sync.dma_start(out=d_dbg_ov[:, :, :], in_=ov[:, :, :])
            esb = work.tile([1, 64], f32)
            nc.scalar.activation(out=esb, in_=ps_sc[0:1, :], func=Exp,
                                 bias=sb_nbv[0:1, :], scale=-1.0)
            nc.vector.tensor_scalar_add(out=esb, in0=esb, scalar1=1.0)
            osb = work.tile([1, 64], f32)
            nc.vector.reciprocal(osb, esb)
            nc.sync.dma_start(out=d_out[:, :], in_=osb)

    nc.compile()
    return nc


# ---------------------------------------------------------------------------
# host-side input prep
# ---------------------------------------------------------------------------

def _prep(inputs):
    global BF16
    import ml_dtypes
    BF16 = ml_dtypes.bfloat16

    enc_state = np.asarray(inputs["enc_state"], dtype=np.float32)
    mb = np.asarray(inputs["memory_bank"], dtype=np.float32)      # [S, B, ENC]
    tgt = np.asarray(inputs["tgt"], dtype=np.float32)             # [T, B, WORD]
    lens = np.asarray(inputs["memory_lengths"]).astype(np.int64)  # [B]
    Wq = np.asarray(inputs["Wq"], dtype=np.float32)
    bq = np.asarray(inputs["bq"], dtype=np.float32)
    Wc = np.asarray(inputs["Wc"], dtype=np.float32)
    v_w = np.asarray(inputs["v_w"], dtype=np.float32)
    W_enc_out = np.asarray(inputs["W_enc_out"], dtype=np.float32)
    b_enc_out = np.asarray(inputs["b_enc_out"], dtype=np.float32)
    W_enc_ctx = np.asarray(inputs["W_enc_ctx"], dtype=np.float32)
    b_enc_ctx = np.asarray(inputs["b_enc_ctx"], dtype=np.float32)
    W_cw = np.asarray(inputs["W_cw"], dtype=np.float32)
    b_cw = np.asarray(inputs["b_cw"], dtype=np.float32)
    w_vrank = np.asarray(inputs["w_vrank"], dtype=np.float32)
    b_vrank = np.asarray(inputs["b_vrank"], dtype=np.float32)

    Lb = tuple(_ceil128(int(l)) for l in lens)
    cum = [0]
    for b in range(B):
        cum.append(cum[-1] + Lb[b])
    SL = cum[-1]
    Sb = [l // 128 for l in Lb]
    cumS = [0]
    for b in range(B):
        cumS.append(cumS[-1] + Sb[b])
    NS = cumS[-1]

    mbT = np.zeros([4, 128, SL], dtype=BF16)
    mbN = np.zeros([NS, 128, ENC], dtype=np.float32)
    for b in range(B):
        seg = mb[:Lb[b], b, :]                       # [Lb, ENC]
        mbT[:, :, cum[b]:cum[b + 1]] = seg.T.reshape(4, 128, Lb[b]).astype(BF16)
        mbN[cumS[b]:cumS[b + 1]] = seg.reshape(Sb[b], 128, ENC)

    wcT = np.ascontiguousarray(Wc.T.reshape(4, 128, ENC)).astype(BF16)
    wqT = np.ascontiguousarray(Wq.T.reshape(4, 128, ENC))
    wcwT = np.ascontiguousarray(W_cw.T.reshape(4, 128, WORD))
    wecT = np.ascontiguousarray(W_enc_ctx.T.reshape(4, 128, WORD))
    weoT = np.ascontiguousarray(W_enc_out.T.reshape(8, 128, WORD))

    enc_hidden = np.concatenate([enc_state[0], enc_state[1]], axis=-1)  # [B, 1024]
    ehT = enc_hidden.T                                                  # [1024, B]
    eh = np.ascontiguousarray(np.repeat(ehT, TL, axis=1).reshape(8, 128, 64))

    vsh = np.zeros([4, 128, 63], dtype=BF16)
    for dc in range(4):
        vsh[dc, :, 31] = v_w[dc * 128:(dc + 1) * 128].astype(BF16)

    vr = np.ascontiguousarray(w_vrank.reshape(4, 128).T)
    bq_t = np.ascontiguousarray(bq.reshape(4, 128).T)
    bw_t = np.ascontiguousarray((b_enc_out + b_enc_ctx + b_cw).reshape(4, 128).T)
    nbv = np.array([[-float(b_vrank)]], dtype=np.float32)

    msk = np.zeros([64, 512], dtype=np.float32)
    for b in range(B):
        msk[b * TL:(b + 1) * TL, int(min(max(lens[b], 0), 512)):] = NEG

    id64 = np.eye(64, dtype=np.float32)

    common = {
        "mbT": mbT, "mbN": mbN, "wcT": wcT, "wqT": wqT, "wcwT": wcwT,
        "wecT": wecT, "weoT": weoT, "eh": eh, "vsh": vsh, "vr": vr,
        "bq": bq_t, "bw": bw_t, "nbv": nbv, "msk": msk, "id64": id64,
    }

    in_maps = []
    for c in range(NCORES):
        # tg[kc, p, j] with j = b*8 + tl for t_global = 8c + tl
        x = tgt[c * TL:(c + 1) * TL]                 # [TL, B, WORD]
        x2 = np.ascontiguousarray(x.transpose(2, 1, 0).reshape(4, 128, 64))
        m = dict(common)
        m["tg"] = x2
        in_maps.append(m)
    return Lb, in_maps


_NC_CACHE = {}


def _get_nc(Lb):
    nc = _NC_CACHE.get(Lb)
    if nc is None:
        nc = _build_nc(Lb)
        _NC_CACHE[Lb] = nc
    return nc


def _assemble(results):
    full = np.zeros([B, T, 1], dtype=np.float32)
    for c in range(NCORES):
        out = np.asarray(results[c]["scores"]).reshape(64)
        for b in range(B):
            full[b, c * TL:(c + 1) * TL, 0] = out[b * TL:(b + 1) * TL]
    return full


def kernel(**inputs):
    from concourse.bass_utils import run_bass_kernel_spmd

    Lb, in_maps = _prep(inputs)
    nc = _get_nc(Lb)
    res = run_bass_kernel_spmd(nc, in_maps, core_ids=list(range(NCORES)))
    return _assemble(res.results)


# -- helper for test.py: build a reusable jitted runner (timing loops) -------

def make_runner(**inputs):
    """Returns (run_once, time_reps). The shard_map'ed executable is built
    ONCE (one neuronx compile); repeat calls measure steady-state
    dispatch+execute time with inputs already resident on-device."""
    import jax
    import numpy as np
    from jax.experimental.shard_map import shard_map
    from jax.sharding import Mesh, NamedSharding, PartitionSpec
    from concourse import bass2jax, mybir
    from concourse.bass2jax import (
        _bass_exec_p, install_neuronx_cc_hook, partition_id_tensor,
    )

    install_neuronx_cc_hook()
    Lb, in_maps = _prep(inputs)
    nc = _get_nc(Lb)
    pid_name = nc.partition_id_tensor.name if nc.partition_id_tensor else None

    in_names, out_names, out_avals, zero_outs = [], [], [], []
    for alloc in nc.m.functions[0].allocations:
        import concourse.mybir as mybir_
        if not isinstance(alloc, mybir_.MemoryLocationSet):
            continue
        name = alloc.memorylocations[0].name
        if alloc.kind == "ExternalInput":
            if name != pid_name:
                in_names.append(name)
        elif alloc.kind == "ExternalOutput":
            shape = tuple(alloc.tensor_shape)
            dtype = mybir_.dt.np(alloc.dtype)
            out_names.append(name)
            out_avals.append(jax.core.ShapedArray(shape, dtype))
            zero_outs.append(np.zeros(shape, dtype))
    n_params = len(in_names)
    n_outs = len(out_avals)
    all_in_names = list(in_names) + list(out_names)
    if pid_name is not None:
        all_in_names.append(pid_name)
    donate = tuple(range(n_params, n_params + n_outs))

    def _body(*args):
        operands = list(args)
        if pid_name is not None:
            operands.append(partition_id_tensor())
        outs = _bass_exec_p.bind(
            *operands,
            out_avals=tuple(out_avals),
            in_names=tuple(all_in_names),
            out_names=tuple(out_names),
            lowering_input_output_aliases=(),
            sim_require_finite=True,
            sim_require_nnan=True,
            nc=nc,
        )
        return tuple(outs)

    devices = jax.devices()[:NCORES]
    mesh = Mesh(np.asarray(devices), ("core",))
    in_specs = (PartitionSpec("core"),) * (n_params + n_outs)
    out_specs = (PartitionSpec("core"),) * n_outs
    sharded = jax.jit(
        shard_map(_body, mesh=mesh, in_specs=in_specs, out_specs=out_specs,
                  check_rep=False),
        donate_argnums=donate, keep_unused=True,
    )
    concat_in = [
        np.concatenate([np.asarray(in_maps[c][name]) for c in range(NCORES)], axis=0)
        for name in in_names
    ]
    shard = NamedSharding(mesh, PartitionSpec("core"))
    concat_in_dev = [jax.device_put(a, shard) for a in concat_in]
    zshapes = [(NCORES * z.shape[0], *z.shape[1:]) for z in zero_outs]
    zdtypes = [z.dtype for z in zero_outs]

    def _zeros_dev():
        return [jax.device_put(np.zeros(s, d), shard)
                for s, d in zip(zshapes, zdtypes)]

    def run_once():
        outs = sharded(*concat_in_dev, *_zeros_dev())
        return [
            {name: np.asarray(outs[i]).reshape(NCORES, *out_avals[i].shape)[c]
             for i, name in enumerate(out_names)}
            for c in range(NCORES)
        ]

    def time_reps(reps=50):
        import time
        outs = sharded(*concat_in_dev, *_zeros_dev())   # warm
        jax.block_until_ready(outs)
        zs = [_zeros_dev() for _ in range(reps)]
        t0 = time.perf_counter()
        all_outs = []
        for r in range(reps):
            all_outs.append(sharded(*concat_in_dev, *zs[r]))
        jax.block_until_ready(all_outs)
        dt = (time.perf_counter() - t0) / reps
        return dt

    return run_once, time_reps
